# revision 40
# baseline (speedup 1.0000x reference)
"""Multi-head self-attention (B=2, T=2048, C=1024, H=16) on 8 trn2 cores.

Sharding: core c -> batch b = c//4, heads 4*(c%4) .. 4*(c%4)+3.
Each core: QKV projection for its 4 heads, causal attention in S^T layout
(keys on partitions), partial output projection over its heads' rows of Wo.
Host sums the 4 partials per batch element and adds bo.

All matmuls run in bfloat16 (full PE rate, half the DMA/SBUF traffic of
fp32r); PSUM accumulation is fp32.

The emission is slab-major so every engine stays busy: for each 512-column
block s, project q/k block s and v chunks 4s..4s+3, then run the pair-0
attention slab (whose deps are exactly those blocks). Pair 1 follows the
same pattern with its q/k blocks, and the output projection is interleaved
into later pair-1 slabs as PE filler. Softmax division is fused per-slab
(copy out of PSUM, reciprocal, gpsimd partition-broadcast, multiply), all
off the PE's critical path.
"""
import sys

sys.path.insert(0, "/opt/trn_rl_repo")

import numpy as np
import ml_dtypes

BF16 = ml_dtypes.bfloat16

B, T, C, H = 2, 2048, 1024, 16
HD = C // H            # 64
NCORES = 8
HPC = H // (NCORES // B)   # heads per core = 4
QB = 128               # q block (columns of S^T)
KB = 128               # k chunk (partitions of S^T)
NJ = T // KB           # 16
NI = T // QB           # 16
SLAB = 512             # q columns processed per attention pass
NSLAB = T // SLAB      # 4
BPS = SLAB // QB       # q blocks per slab = 4
CI = C // 128          # 8 contraction chunks for projections
SCALE = HD ** -0.5

_cache = {}


def _build_plan(mask_bool: np.ndarray):
    """mask_bool: [T, T] (q, k). Returns per (j, i) block types and tiles.

    type 0 = all valid (no mask work), 1 = all masked (skip), 2 = mixed.
    Tiles are stored transposed to match S^T ([k_local, q_local])."""
    btype = np.zeros((NJ, NI), dtype=np.int32)
    tidx = np.full((NJ, NI), -1, dtype=np.int32)
    tiles = []
    tile_map = {}
    for j in range(NJ):
        for i in range(NI):
            sub = mask_bool[i * QB:(i + 1) * QB, j * KB:(j + 1) * KB]
            if sub.all():
                btype[j, i] = 0
            elif not sub.any():
                btype[j, i] = 1
            else:
                btype[j, i] = 2
                key = sub.tobytes()
                if key not in tile_map:
                    tile_map[key] = len(tiles)
                    tiles.append(sub.T.astype(BF16))
                tidx[j, i] = tile_map[key]
    if not tiles:
        tiles.append(np.ones((KB, QB), dtype=BF16))
    return btype, tidx, np.stack(tiles)


def _build_program(btype, tidx, n_tiles, apply_qk_bias, apply_v_bias):
    import concourse.bass as bass
    import concourse.tile as tile
    import concourse.mybir as mybir
    from concourse import bacc

    F32 = mybir.dt.float32
    B16 = mybir.dt.bfloat16
    AF = mybir.ActivationFunctionType
    MULT = mybir.AluOpType.mult

    nc = bacc.Bacc("TRN2", target_bir_lowering=False, debug=False)
    xt_d = nc.dram_tensor("xt", [C, T], B16, kind="ExternalInput").ap()
    wqk_d = nc.dram_tensor("wqk", [C, 4 * 128], B16, kind="ExternalInput").ap()
    wv_d = nc.dram_tensor("wv", [C, HPC * HD], B16, kind="ExternalInput").ap()
    wo_d = nc.dram_tensor("wo", [HPC * HD, C], B16, kind="ExternalInput").ap()
    mask_d = nc.dram_tensor("masks", [n_tiles, KB, QB], B16,
                            kind="ExternalInput").ap()
    bqk_d = nc.dram_tensor("bqk", [128, 4], F32, kind="ExternalInput").ap()
    bv_d = nc.dram_tensor("bv", [128, 2], F32, kind="ExternalInput").ap()
    zero_d = nc.dram_tensor("zeros", [64, T], B16, kind="ExternalInput").ap()
    out_d = nc.dram_tensor("out", [T, C], B16, kind="ExternalOutput").ap()

    with tile.TileContext(nc) as tc:
        with tc.tile_pool(name="weights", bufs=1) as wpool, \
             tc.tile_pool(name="acts", bufs=1) as apool, \
             tc.tile_pool(name="xtp", bufs=1) as xtp, \
             tc.tile_pool(name="pp", bufs=2, space="PSUM") as pp, \
             tc.tile_pool(name="psattn", bufs=1, space="PSUM") as sp, \
             tc.tile_pool(name="psout", bufs=1, space="PSUM") as op, \
             tc.tile_pool(name="ptp", bufs=5) as ptp, \
             tc.tile_pool(name="divp", bufs=2) as divp, \
             tc.tile_pool(name="osb", bufs=3) as osb:
            # ---- resident SBUF tensors ----
            wo = wpool.tile([128, 2, C], B16)             # head-pair chunks
            masks = wpool.tile([128, n_tiles * QB], B16)
            bqk = wpool.tile([128, 4], F32)
            bv = wpool.tile([128, 2], F32)
            # q tiles hold (q_hA | q_hB) on partitions 0-63 / 64-127.
            # k is stored zero-padded per head (other head's partitions are
            # zero) so S matmuls present K=128 to the PE.
            qp = [apool.tile([128, T], B16, tag=f"qp{i}", name=f"qp{i}")
                  for i in range(2)]
            kz = [apool.tile([128, T], B16, tag=f"kz{i}", name=f"kz{i}")
                  for i in range(4)]          # index = 2*pair + head
            vaug = apool.tile([128, NJ, HPC * (HD + 1)], B16)
            # per-slab attn tiles: tile-granular dependency tracking would
            # otherwise make every output-projection group wait on the
            # latest division write
            attn = [[apool.tile([128, SLAB], B16, tag=f"attn{p}_{sl}",
                                name=f"attn{p}_{sl}") for sl in range(NSLAB)]
                    for p in range(2)]
            xt = xtp.tile([128, CI, T], B16)       # x^T, c_in chunked
            wqk = xtp.tile([128, CI, 512], B16)
            wv = xtp.tile([128, CI, HPC * HD], B16)

            # ---- DMA, ordered to match consumption ----
            qt4 = T // 4
            for ci in range(CI):
                nc.sync.dma_start(wqk[:, ci, :],
                                  wqk_d[ci * 128:(ci + 1) * 128, :])
                nc.sync.dma_start(
                    xt[:, ci, 0:qt4], xt_d[ci * 128:(ci + 1) * 128, 0:qt4])
            for ci in range(CI):
                nc.sync.dma_start(wv[:, ci, :],
                                  wv_d[ci * 128:(ci + 1) * 128, :])
            for p in range(2):
                nc.sync.dma_start(kz[2 * p][64:128, :], zero_d)
                nc.sync.dma_start(kz[2 * p + 1][0:64, :], zero_d)
            nc.sync.dma_start(bqk[:], bqk_d)
            for t in range(n_tiles):
                nc.sync.dma_start(masks[:, t * QB:(t + 1) * QB], mask_d[t])
            nc.sync.dma_start(bv[:], bv_d)
            for ci in range(CI):
                nc.sync.dma_start(
                    xt[:, ci, qt4:2 * qt4],
                    xt_d[ci * 128:(ci + 1) * 128, qt4:2 * qt4])
            for qn in range(2, 4):
                for ci in range(CI):
                    nc.sync.dma_start(
                        xt[:, ci, qn * qt4:(qn + 1) * qt4],
                        xt_d[ci * 128:(ci + 1) * 128,
                             qn * qt4:(qn + 1) * qt4])
            nc.sync.dma_start(wo[:, 0, :], wo_d[0:128, :])
            nc.sync.dma_start(wo[:, 1, :], wo_d[128:256, :])

            # clock-ramp warmup: dep-free matmuls on zeroed SBUF keep the
            # HAM activity monitor busy while the first DMAs land, so real
            # matmuls start at the boosted clock instead of 0.65-1.2 GHz
            wz = apool.tile([128, 512], B16, tag="warmz", name="warmz")
            nc.vector.memset(wz[:], 0.0)
            wps = pp.tile([128, 512], F32, tag="pp", name="ppwarm")
            for wi in range(20):
                nc.tensor.matmul(wps[:], wz[:, 0:128], wz[:],
                                 start=(wi == 0), stop=(wi == 19))
            wsink = wpool.tile([1, 8], F32)
            nc.vector.tensor_copy(wsink[:], wps[0:1, 0:8])

            va = vaug[:].rearrange("p j (h d) -> p j h d", h=HPC)
            nc.vector.tensor_copy(
                va[:, :, :, HD:HD + 1],
                nc.const_aps.tensor(1.0, (128, NJ, HPC, 1)))
            # pre-warm the gpsimd PartitionBroadcast library: the first
            # broadcast after other gpsimd op types pays a ~7us
            # LIBRARY_RELOAD that would otherwise stall the attention
            # pipeline
            warm = wpool.tile([64, 4], F32)
            nc.gpsimd.partition_broadcast(warm[:], bqk[0:1, 0:4])
            # ones column for the PE-side partition broadcast (final slab)
            onec = wpool.tile([1, 64], F32)
            nc.vector.tensor_copy(onec[:], nc.const_aps.tensor(1.0, (1, 64)))

            # ---- emission helpers ----
            def emit_qk_group(co, ts):
                sl = slice(ts * 512, (ts + 1) * 512)
                ps = pp.tile([128, 512], F32, tag="pp", name="ppqk")
                for ci in range(CI):
                    nc.tensor.matmul(
                        ps[:], wqk[:, ci, co * 128:(co + 1) * 128],
                        xt[:, ci, sl],
                        start=(ci == 0), stop=(ci == CI - 1))
                pair, is_k = co // 2, co % 2
                if is_k:
                    dsts = [(kz[2 * pair][0:64, sl], ps[0:64, :],
                             bqk[0:64, co:co + 1]),
                            (kz[2 * pair + 1][64:128, sl],
                             ps[64:128, :], bqk[64:128, co:co + 1])]
                else:
                    dsts = [(qp[pair][:, sl], ps[:], bqk[:, co:co + 1])]
                for dst_ap, src_ap, b_ap in dsts:
                    # on ACT: DVE is loaded with division/mask/output work
                    if apply_qk_bias:
                        nc.scalar.activation(dst_ap, src_ap, AF.Identity,
                                             bias=b_ap, scale=1.0)
                    else:
                        nc.scalar.copy(dst_ap, src_ap)

            def emit_v_group(tj):
                ps = pp.tile([128, 512], F32, tag="pp", name="ppv")
                for ci in range(CI):
                    nc.tensor.matmul(
                        ps[:, 0:HPC * HD],
                        xt[:, ci, tj * 128:(tj + 1) * 128],
                        wv[:, ci, :],
                        start=(ci == 0), stop=(ci == CI - 1))
                nc.vector.tensor_copy(
                    va[:, tj, :, 0:HD],
                    ps[:, 0:HPC * HD].rearrange("p (h d) -> p h d", h=HPC))

            outq = []      # pending output-projection groups
            flushn = [0]

            def emit_outproj_group(flush=False):
                ts, half = outq.pop(0)
                n0 = half * 512
                ps = pp.tile([128, 512], F32, tag="pp", name="ppo")
                sb, tb = divmod(ts, BPS)
                for pair in range(2):
                    nc.tensor.matmul(
                        ps[:], attn[pair][sb][:, tb * 128:(tb + 1) * 128],
                        wo[:, pair, n0:n0 + 512],
                        start=(pair == 0), stop=(pair == 1))
                ot = osb.tile([128, 512], B16, tag="ot", name="ot")
                # during attention ACT is saturated with exp, so drained
                # groups copy on DVE; at the final flush both engines are
                # free — alternate so neither paces the tail
                if flush and flushn[0] % 2 == 0:
                    nc.scalar.copy(ot[:], ps[:])
                else:
                    nc.vector.tensor_copy(ot[:], ps[:])
                flushn[0] += 1
                nc.sync.dma_start(
                    out_d[ts * 128:(ts + 1) * 128, n0:n0 + 512], ot[:])

            def emit_attn_slab(pair, s):
                q_t = qp[pair]
                i_lo, i_hi = s * BPS, (s + 1) * BPS
                chunks = []
                for j in range(NJ):
                    live = [i for i in range(i_lo, i_hi) if btype[j, i] != 1]
                    if live:
                        chunks.append((j, min(live), max(live)))
                out_ps = [op.tile([HD + 1, SLAB], F32, tag=f"outps{_hl}",
                                  name=f"outps{_hl}", bufs=1)
                          for _hl in range(2)]
                written = np.zeros(BPS, dtype=bool)
                for cn, (j, i0, i1) in enumerate(chunks):
                    n_cols = (i1 - i0 + 1) * QB
                    r0 = i0 - i_lo
                    # S^T for both heads into the two banks of one psum
                    # tile; one exp covers both
                    sps = sp.tile([128, 2, SLAB], F32,
                                  tag="sst", name="sst", bufs=2)
                    for hl in range(2):
                        nc.tensor.matmul(
                            sps[:, hl, 0:n_cols],
                            kz[2 * pair + hl][:, j * KB:(j + 1) * KB],
                            q_t[:, i0 * QB:i0 * QB + n_cols],
                            start=True, stop=True)
                    pt = ptp.tile([128, 2, SLAB], B16, tag="pt", name="pt")
                    nc.scalar.activation(pt[:, :, 0:n_cols],
                                         sps[:, :, 0:n_cols],
                                         AF.Exp, scale=SCALE)
                    for i in range(i0, i1 + 1):
                        rel = (i - i0) * QB
                        if btype[j, i] == 2:
                            ti = tidx[j, i]
                            m2 = masks[:, ti * QB:(ti + 1) * QB] \
                                .unsqueeze(1).broadcast_to([128, 2, QB])
                            # on DVE (2x for bf16), keeping gpsimd
                            # single-library (PartitionBroadcast only)
                            nc.vector.tensor_tensor(
                                out=pt[:, :, rel:rel + QB],
                                in0=pt[:, :, rel:rel + QB],
                                in1=m2, op=MULT)
                        elif btype[j, i] == 1:
                            nc.vector.memset(pt[:, :, rel:rel + QB], 0.0)
                    # PV accumulation (runs are <= 512 so no bank crossing;
                    # split only on first-write transitions)
                    segs = []
                    c = r0 * QB
                    end = (i1 - i_lo + 1) * QB
                    while c < end:
                        st = written[c // QB]
                        cc = c + QB
                        while cc < end and written[cc // QB] == st:
                            cc += QB
                        segs.append((c, cc, not st))
                        c = cc
                    last = cn == len(chunks) - 1
                    for hl in range(2):
                        hh = 2 * pair + hl
                        for (c0, c1, st_flag) in segs:
                            nc.tensor.matmul(
                                out_ps[hl][:, c0:c1],
                                vaug[:, j, hh * (HD + 1):(hh + 1) * (HD + 1)],
                                pt[:, hl, c0 - r0 * QB:c1 - r0 * QB],
                                start=st_flag, stop=last,
                                skip_group_check=True)
                    for rr in range(r0, i1 - i_lo + 1):
                        written[rr] = True
                    # drain one pending output-projection group per chunk
                    # as PE filler (pair-1 slabs only). Skip the first two
                    # boundaries (the previous slab's division mult — which
                    # the group's LDWEIGHTS waits on — is still in the DVE
                    # queue), and hold everything in the last slab so the
                    # flush covers the final division chain.
                    if outq and (pair, s) != (1, NSLAB - 1) \
                            and 3 <= cn < len(chunks) - 1:
                        emit_outproj_group()
                # in the final slab, drain all ready output-projection
                # groups BEFORE emitting the division: attn reads are
                # guarded by a counting semaphore incremented by every
                # division mult, so groups emitted after this division
                # would wait for it even though they read earlier slabs
                final = (pair, s) == (1, NSLAB - 1)
                if final:
                    while outq:
                        emit_outproj_group(flush=True)
                # fused softmax division: copy the accumulator out of PSUM
                # (releasing it for the next slab), then reciprocal of the
                # denominator row, gpsimd partition-broadcast, multiply.
                # For the final slab, high priority hoists the chain ahead
                # of the cover groups' copies in the DVE queue, and the
                # und indirection is skipped (no one needs the PSUM bank).
                import contextlib
                # hoist the division chain ahead of drained-group copies
                # in the DVE queue on every slab: the next slab's drains
                # wait on this division's mult via the counting semaphore
                hp = tc.high_priority(offset=120 if final else 40)
                with hp:
                    recb_ps = None
                    if final:
                        recb_ps = sp.tile([128, 2, SLAB], F32,
                                          tag="sst", name="sstb", bufs=2)
                    for hl in range(2):
                        # und copy releases the PSUM accumulator early and
                        # keeps the division mult single-PSUM-operand; hl0
                        # on ACT (idle window after the slab's last exp),
                        # hl1 on DVE, so they run in parallel
                        und = divp.tile([HD, SLAB], F32, tag="und",
                                        name="und")
                        if hl == 0 or final:
                            nc.scalar.copy(und[:], out_ps[hl][0:HD, :])
                        else:
                            nc.vector.tensor_copy(und[:],
                                                  out_ps[hl][0:HD, :])
                        src0 = und[:]
                        srow = divp.tile([1, SLAB], F32, tag="srow",
                                         name="srow")
                        nc.vector.tensor_copy(srow[:],
                                              out_ps[hl][HD:HD + 1, :])
                        rec = divp.tile([1, SLAB], F32, tag="rec",
                                        name="rec")
                        nc.vector.reciprocal_approx_fast(rec[:], srow[:])
                        dst = attn[pair][s][64 * hl:64 * hl + 64, :]
                        if final:
                            # PE-side broadcast: ones-column matmul into a
                            # free sps bank — the gpsimd path costs ~2us of
                            # chain latency right when the PE is idle
                            nc.tensor.matmul(
                                recb_ps[0:64, hl, :],
                                onec[:], rec[:],
                                start=True, stop=True)
                            in1 = recb_ps[0:64, hl, :]
                        else:
                            recb = divp.tile([64, SLAB], F32, tag="recb",
                                             name="recb")
                            nc.gpsimd.partition_broadcast(recb[:], rec[:])
                            in1 = recb[:]
                        nc.vector.tensor_tensor(out=dst, in0=src0,
                                                in1=in1, op=MULT)
                        if apply_v_bias:
                            nc.vector.tensor_scalar(
                                out=dst, in0=dst,
                                scalar1=bv[64 * hl:64 * hl + 64,
                                           pair:pair + 1],
                                scalar2=None, op0=mybir.AluOpType.add)

            # ---- slab-major emission ----
            for s in range(NSLAB):
                emit_qk_group(0, s)
                emit_qk_group(1, s)
                for tj in range(4 * s, 4 * s + 4):
                    emit_v_group(tj)
                emit_attn_slab(0, s)
            # pair-1 qk groups staggered a half-step early so the PE has
            # projection work while each division chain runs
            emit_qk_group(2, 0)
            emit_qk_group(3, 0)
            emit_qk_group(2, 1)
            for s in range(NSLAB):
                emit_attn_slab(1, s)
                for ts in range(s * BPS, (s + 1) * BPS):
                    outq.append((ts, 0))
                    outq.append((ts, 1))
                if s + 1 < NSLAB:
                    emit_qk_group(3, s + 1)
                if s + 2 < NSLAB:
                    emit_qk_group(2, s + 2)
            while outq:
                emit_outproj_group(flush=True)

    nc.compile()
    return nc


def _get_program(mask_bool, apply_qk_bias, apply_v_bias):
    key = (mask_bool.tobytes(), apply_qk_bias, apply_v_bias)
    if key not in _cache:
        btype, tidx, tiles = _build_plan(mask_bool)
        nc = _build_program(btype, tidx, len(tiles), apply_qk_bias,
                            apply_v_bias)
        _cache[key] = (nc, tiles)
    return _cache[key]


def kernel(x, attention_mask, Wqkv, bqkv, Wo, bo, _trace=False):
    from concourse.bass_utils import run_bass_kernel_spmd

    x = np.asarray(x, dtype=np.float32)
    mask_bool = np.asarray(attention_mask)[0, 0] != 0
    Wqkv = np.asarray(Wqkv, dtype=np.float32)
    bqkv = np.asarray(bqkv, dtype=np.float32)
    Wo = np.asarray(Wo, dtype=np.float32)
    bo = np.asarray(bo, dtype=np.float32)

    apply_qk_bias = bool(np.any(bqkv[:2 * C]))
    apply_v_bias = bool(np.any(bqkv[2 * C:]))
    nc, tiles = _get_program(mask_bool, apply_qk_bias, apply_v_bias)

    xts = [np.ascontiguousarray(x[b].T).astype(BF16) for b in range(B)]
    zeros = np.zeros((64, T), dtype=BF16)
    in_maps = []
    for c in range(NCORES):
        b, g = divmod(c, NCORES // B)
        hs = [HPC * g + i for i in range(HPC)]
        # wqk column chunks: [q_h0|q_h1, k_h0|k_h1, q_h2|q_h3, k_h2|k_h3]
        cols, bias_cols = [], []
        for pair in range(2):
            ha, hb = hs[2 * pair], hs[2 * pair + 1]
            for base in (0, C):  # q then k offset in Wqkv columns
                cols.append(Wqkv[:, base + ha * HD:base + (ha + 1) * HD])
                cols.append(Wqkv[:, base + hb * HD:base + (hb + 1) * HD])
                bias_cols.append(np.concatenate([
                    bqkv[base + ha * HD:base + (ha + 1) * HD],
                    bqkv[base + hb * HD:base + (hb + 1) * HD]]))
        wqk_c = np.concatenate(cols, axis=1).astype(BF16)
        bqk_c = np.stack(bias_cols, axis=1).astype(np.float32)
        wv_c = np.concatenate(
            [Wqkv[:, 2 * C + h * HD:2 * C + (h + 1) * HD] for h in hs],
            axis=1).astype(BF16)
        wo_c = np.concatenate(
            [Wo[h * HD:(h + 1) * HD, :] for h in hs], axis=0).astype(BF16)
        bv_c = np.zeros((128, 2), dtype=np.float32)
        for pair in range(2):
            ha, hb = hs[2 * pair], hs[2 * pair + 1]
            bv_c[0:HD, pair] = bqkv[2 * C + ha * HD:2 * C + (ha + 1) * HD]
            bv_c[HD:128, pair] = bqkv[2 * C + hb * HD:2 * C + (hb + 1) * HD]
        in_maps.append({
            "xt": xts[b], "wqk": wqk_c, "wv": wv_c, "wo": wo_c,
            "masks": tiles, "bqk": bqk_c, "bv": bv_c, "zeros": zeros,
        })

    kwargs = {}
    if _trace:
        kwargs = dict(trace=True, trace_cores=[0])
    res = run_bass_kernel_spmd(nc, in_maps, core_ids=list(range(NCORES)),
                               **kwargs)
    out = np.empty((B, T, C), dtype=np.float32)
    gpb = NCORES // B
    for b in range(B):
        acc = res.results[b * gpb]["out"].astype(np.float32)
        for g in range(1, gpb):
            acc = acc + res.results[b * gpb + g]["out"].astype(np.float32)
        out[b] = acc + bo
    if _trace:
        kernel._last_results = res
    return out
